# revision 15
# baseline (speedup 1.0000x reference)
"""Trainium2 Bass kernel for the CustomJacobiLayer problem.

Computes out[b,j] = sum_{i,d} P_d(tanh(x[b,i])) * coef[j,i,d]
with P_d the Jacobi(alpha=1,beta=1) polynomials, d=0..7.

Strategy (8 NeuronCores, data-parallel over batch):
  - Each core owns 512 of the 4096 batch rows; coef is replicated.
  - Host-side: the three-term Jacobi recurrence
        p_d = K1_d * t * p_{d-1} - K3_d * p_{d-2}     (K2_d == 0 for a==b)
    is rescaled with q_d = p_d / s_d, s_d = K1_d * s_{d-1}, so the device
    recurrence has a unit leading coefficient:
        q_d = t * q_{d-1} - g_d * q_{d-2}
    The scales s_d are folded into coef (in float64), so only two fp16
    VectorE ops per order are needed on-chip.
  - The d=0 term is P_0 == 1, i.e. a rank-1 bias sum_i coef[j,i,0]; it is
    computed on the host and added after the gather.
  - Device: ScalarE tanh (f32 -> fp16), VectorE recurrence chain (fp16,
    2x perf mode), 112 accumulating TensorE matmuls (fp16, N=512) into
    4 PSUM banks, PSUM DMA'd straight to HBM.

Numerics (vs f64 reference, measured in emulation): max err / max|out|
~2.5e-3 -- fp16 matmul inputs, fp32 PSUM accumulation.
"""

import numpy as np

ORDER = 7
ALPHA = 1.0
BETA = 1.0
B_FULL, I_DIM, O_DIM = 4096, 512, 512
N_CORES = 8
BS = B_FULL // N_CORES  # 512 batch rows per core
P = 128                 # SBUF partitions
IC = I_DIM // P         # 4 i-chunks
BT = BS // P            # 4 batch tiles per core


def _recurrence_constants():
    """K1/K3 per reference, rescaled so q_d = t*q_{d-1} - g_d*q_{d-2}."""
    k1 = np.zeros(ORDER + 1, dtype=np.float64)
    k3 = np.zeros(ORDER + 1, dtype=np.float64)
    a, b = ALPHA, BETA
    for i in range(2, ORDER + 1):
        k1[i] = (2 * i + a + b) * (2 * i + a + b - 1) / (2 * i * (i + a + b))
        k3[i] = (
            (i + a - 1) * (i + b - 1) * (2 * i + a + b)
            / (i * (i + a + b) * (2 * i + a + b - 2))
        )
    s = np.zeros(ORDER + 1, dtype=np.float64)
    s[0] = 1.0
    s[1] = 0.5 * (a + b + 2.0)  # p_1 = s_1 * t  (the -(a-b)/2 term is 0)
    for d in range(2, ORDER + 1):
        s[d] = k1[d] * s[d - 1]
    g = np.zeros(ORDER + 1, dtype=np.float64)
    for d in range(2, ORDER + 1):
        g[d] = k3[d] * s[d - 2] / s[d]
    return s, g


_S, _G = _recurrence_constants()

_NC_CACHE = {}


def _build_bass():
    from contextlib import ExitStack
    from concourse import bacc, bass, tile, mybir

    nc = bacc.Bacc(
        "TRN2",
        target_bir_lowering=False,
        debug=False,
        num_devices=1,
    )
    f32 = mybir.dt.float32
    f16 = mybir.dt.float16

    xT = nc.dram_tensor("xT", [I_DIM, BS], f16, kind="ExternalInput")
    cf = nc.dram_tensor("cf", [ORDER, I_DIM, O_DIM], f16, kind="ExternalInput")
    out = nc.dram_tensor("out", [BS, O_DIM], f32, kind="ExternalOutput")

    FD = IC * BS  # 2048: all 4 i-chunks side by side on the free axis

    with tile.TileContext(nc) as tc, ExitStack() as ctx:
        pool = ctx.enter_context(tc.tile_pool(name="main", bufs=1))
        psum = ctx.enter_context(
            tc.tile_pool(name="psum", bufs=1, space=bass.MemorySpace.PSUM)
        )

        # PE warm-up: ~3.4us of throwaway matmuls so the HAM clock gate is
        # released before the first real matmul issues.
        wtile = pool.tile([P, O_DIM], f16, tag="warm")
        nc.vector.memset(wtile[:], 0.5)
        ps_w = psum.tile([P, O_DIM], f32, tag="ps_w", name="ps_w")
        N_WARM = 6
        for w in range(N_WARM):
            nc.tensor.matmul(
                ps_w[:], wtile[:, 0:P], wtile[:],
                start=(w == 0), stop=(w == N_WARM - 1),
            )

        # x in (one DMA per i-chunk), tanh -> fp16 per chunk: the first
        # d=1 matmuls only need t[:, ic, :], so get those ready early.
        # All input DMAs are issued from GpSimd (Sync is stuck in the Tile
        # preamble for ~7us), in priority order: x0, cf1, x1..x3, cf2..cf7.
        xt = pool.tile([P, IC, BS], f16, tag="x")
        t = pool.tile([P, IC, BS], f16, tag="t")
        cfs = [None] * (ORDER + 1)

        def load_cf(d):
            c_t = pool.tile([P, IC, O_DIM], f16, tag=f"cf{d}", name=f"cf{d}")
            nc.gpsimd.dma_start(
                c_t[:], cf[d - 1].rearrange("(ic p) j -> p ic j", p=P)
            )
            cfs[d] = c_t

        def load_x(ic, dma_engine):
            dma_engine.dma_start(xt[:, ic, :], xT[ic * P:(ic + 1) * P, :])
            nc.scalar.activation(
                t[:, ic, :], xt[:, ic, :], mybir.ActivationFunctionType.Tanh
            )

        # First-data race: x0 is issued from ScalarE (before its activation
        # table load) while GpSimd issues cf1's first chunks in parallel.
        cf1 = pool.tile([P, IC, O_DIM], f16, tag="cf1", name="cf1")
        cf1_src = cf[0].rearrange("(ic p) j -> p ic j", p=P)
        load_x(0, nc.scalar)
        nc.gpsimd.dma_start(cf1[:, 0:2, :], cf1_src[:, 0:2, :])
        cfs[1] = cf1
        load_x(1, nc.gpsimd)
        load_x(2, nc.gpsimd)
        load_x(3, nc.gpsimd)
        nc.gpsimd.dma_start(cf1[:, 2:, :], cf1_src[:, 2:, :])
        for d in range(2, ORDER + 1):
            load_cf(d)

        # recurrence chain over the full [128, 2048] plane:
        #   q_1 = t; q_2 = t*t - g_2; q_d = t*q_{d-1} - g_d*q_{d-2}
        # scalar_tensor_tensor only runs at DVE 1x, so the serial chain uses
        # two 2x tensor_tensor ops per order; the scalar multiply
        # w_d = -g_d * q_{d-2} is offloaded to the (otherwise idle) ScalarE
        # one step ahead of when the chain needs it.
        q = [None] * (ORDER + 1)
        q[1] = t
        w = [None] * (ORDER + 1)
        w3 = pool.tile([P, IC, BS], f16, tag="w3")
        nc.vector.tensor_scalar_mul(w3[:], t[:], -float(_G[3]))
        w[3] = w3
        for d in range(2, ORDER + 1):
            m = pool.tile([P, IC, BS], f16, tag=f"m{d}")
            nc.vector.tensor_tensor(m[:], t[:], q[d - 1][:], mybir.AluOpType.mult)
            qd = pool.tile([P, IC, BS], f16, tag=f"q{d}")
            if d == 2:
                # q_0 == 1: scalar add (tensor_scalar runs at DVE 4x)
                nc.vector.tensor_scalar_add(qd[:], m[:], -float(_G[d]))
            else:
                nc.vector.tensor_tensor(
                    qd[:], m[:], w[d][:], mybir.AluOpType.add
                )
            q[d] = qd
            if d + 2 <= ORDER:
                wd = pool.tile([P, IC, BS], f16, tag=f"w{d+2}")
                nc.vector.tensor_scalar_mul(wd[:], qd[:], -float(_G[d + 2]))
                w[d + 2] = wd

        # matmuls: psum[b] += q[d][:, ic*BS+b*128 :+128].T @ cfs[d][:, ic*O :+O]
        ps = [
            psum.tile([P, O_DIM], f32, tag=f"ps{b}", name=f"ps{b}")
            for b in range(BT)
        ]
        for d in range(1, ORDER + 1):
            for ic in range(IC):
                first = d == 1 and ic == 0
                last = d == ORDER and ic == IC - 1
                for b in range(BT):
                    nc.tensor.matmul(
                        ps[b][:],
                        q[d][:, ic, b * P:(b + 1) * P],
                        cfs[d][:, ic, :],
                        start=first,
                        stop=last,
                    )

        # PSUM -> SBUF -> HBM (DMA cannot read PSUM directly); one copy+DMA
        # per batch tile so stores overlap the remaining copies.
        ot = pool.tile([P, BT, O_DIM], f32, tag="o")
        for b in range(BT):
            if b % 2 == 0:
                nc.scalar.copy(ot[:, b, :], ps[b][:])
                nc.scalar.dma_start(out[b * P:(b + 1) * P, :], ot[:, b, :])
            else:
                nc.vector.tensor_copy(ot[:, b, :], ps[b][:])
                nc.sync.dma_start(out[b * P:(b + 1) * P, :], ot[:, b, :])

    nc.compile()
    return nc


def _get_nc():
    if "nc" not in _NC_CACHE:
        _NC_CACHE["nc"] = _build_bass()
    return _NC_CACHE["nc"]


def _host_prep(x, coef):
    """Shard + transform inputs. Returns (in_maps, bias)."""
    x = np.asarray(x, dtype=np.float32)
    coef = np.asarray(coef, dtype=np.float32)

    # [d, i, j] with the recurrence scale folded in, orders 1..7, fp16
    cf_t = coef.astype(np.float64).transpose(2, 1, 0)  # [8, I, O]
    cf_dev = np.ascontiguousarray(
        (cf_t[1:] * _S[1:, None, None]).astype(np.float16)
    )
    # d = 0 term: P_0 == 1  ->  bias[j] = sum_i coef[j, i, 0]
    bias = cf_t[0].sum(axis=0)  # [O] f64

    xT = np.ascontiguousarray(x.T.astype(np.float16))  # [I, B] fp16
    in_maps = [
        {"xT": np.ascontiguousarray(xT[:, c * BS:(c + 1) * BS]), "cf": cf_dev}
        for c in range(N_CORES)
    ]
    return in_maps, bias


def kernel(x, coef):
    from concourse.bass_utils import run_bass_kernel_spmd

    nc = _get_nc()
    in_maps, bias = _host_prep(x, coef)
    res = run_bass_kernel_spmd(nc, in_maps, core_ids=list(range(N_CORES)))
    out = np.concatenate(
        [res.results[c]["out"] for c in range(N_CORES)], axis=0
    ).astype(np.float64)
    out += bias[None, :]
    return out.astype(np.float32)
